# revision 1
# baseline (speedup 1.0000x reference)
"""Trainium2 Bass kernel for a causal multi-head attention layer.

Model: b=2, s=2048, d_model=1024, 16 heads, head_dim=64, pad-index 0.
Sharding over 8 NeuronCores: each core owns 2 heads (128 of the 1024
attention dims) for both batches (head/tensor parallel).  After attention,
an AllToAll redistributes the per-head outputs so each core holds all 1024
attention dims for 1/8 of the sequence positions, where it runs the output
projection locally.  Output rows per core: 256 rows of each batch.
"""

import threading

import numpy as np

B, S, D = 2, 2048, 1024
H, HD = 16, 64
NCORES = 8
LD = D // NCORES          # 128 local attention dims (2 heads)
R = B * S                 # 4096 flattened rows
RC = R // NCORES          # 512 output rows per core
RB = S // NCORES          # 256 rows per batch per core
NKT = S // 128            # 16 key tiles per batch
NCH = D // 128            # 8 contraction chunks of d_model

_cache = {}
_lock = threading.Lock()


def _build_nc():
    import concourse.mybir as mybir
    import concourse.tile as tile
    from concourse import bacc
    from concourse.masks import make_identity
    from contextlib import ExitStack

    f32 = mybir.dt.float32
    bf16 = mybir.dt.bfloat16
    i32 = mybir.dt.int32
    AF = mybir.ActivationFunctionType
    ALU = mybir.AluOpType

    nc = bacc.Bacc(None, target_bir_lowering=False, num_devices=NCORES)

    xT = nc.declare_dram_parameter("xT", [D, R], bf16, isOutput=False)
    wqT = nc.declare_dram_parameter("wqT", [D, LD], bf16, isOutput=False)
    wkT = nc.declare_dram_parameter("wkT", [D, LD], bf16, isOutput=False)
    wvT = nc.declare_dram_parameter("wvT", [D, LD], bf16, isOutput=False)
    woT = nc.declare_dram_parameter("woT", [D, D], bf16, isOutput=False)
    bq = nc.declare_dram_parameter("bq", [LD], f32, isOutput=False)
    bk = nc.declare_dram_parameter("bk", [LD], f32, isOutput=False)
    bv = nc.declare_dram_parameter("bv", [LD], f32, isOutput=False)
    bo = nc.declare_dram_parameter("bo", [D], f32, isOutput=False)
    ids = nc.declare_dram_parameter("ids", [128, B * NKT], i32, isOutput=False)
    out = nc.declare_dram_parameter("out", [RC, D], f32, isOutput=True)

    with ExitStack() as ctx:
        tc = ctx.enter_context(tile.TileContext(nc))
        const = ctx.enter_context(tc.tile_pool(name="const", bufs=1))
        qkp = ctx.enter_context(tc.tile_pool(name="qkp", bufs=2))
        work = ctx.enter_context(tc.tile_pool(name="work", bufs=4))
        est = ctx.enter_context(tc.tile_pool(name="est", bufs=1))
        stg = ctx.enter_context(tc.tile_pool(name="stg", bufs=2))
        spool = ctx.enter_context(tc.tile_pool(name="spool", bufs=2, space="PSUM"))
        opool = ctx.enter_context(tc.tile_pool(name="opool", bufs=4, space="PSUM"))
        dpool = ctx.enter_context(tc.tile_pool(name="dram", bufs=2, space="DRAM"))

        # ---- constants (small weights first so compute can start early) ----
        wqT_sb = const.tile([128, NCH, LD], bf16)
        nc.sync.dma_start(wqT_sb, wqT.ap().rearrange("(c p) d -> p c d", p=128))
        wkT_sb = const.tile([128, NCH, LD], bf16)
        nc.sync.dma_start(wkT_sb, wkT.ap().rearrange("(c p) d -> p c d", p=128))
        wvT_sb = const.tile([128, NCH, LD], bf16)
        nc.sync.dma_start(wvT_sb, wvT.ap().rearrange("(c p) d -> p c d", p=128))
        # x^T loaded as 8 independent contraction-chunk tiles so projection
        # matmuls on chunk c start as soon as chunk c lands
        xTr = xT.ap().rearrange("(c p) r -> c p r", p=128)
        xT_ch = []
        for c in range(NCH):
            xc = const.tile([128, R], bf16, name=f"xc{c}", tag=f"xc{c}")
            nc.sync.dma_start(xc, xTr[c])
            xT_ch.append(xc)
        woT_sb = const.tile([128, NCH, D], bf16)
        nc.sync.dma_start(woT_sb, woT.ap().rearrange("(c p) n -> p c n", p=128))

        bq_col = const.tile([128, 1], f32)
        nc.sync.dma_start(bq_col, bq.ap().rearrange("(p o) -> p o", o=1))
        bk_col = const.tile([128, 1], f32)
        nc.sync.dma_start(bk_col, bk.ap().rearrange("(p o) -> p o", o=1))
        bv_bc = const.tile([128, LD], f32)
        nc.sync.dma_start(bv_bc, bv.ap().partition_broadcast(128))
        bo_bc = const.tile([128, D], f32)
        nc.sync.dma_start(bo_bc, bo.ap().partition_broadcast(128))

        ids_sb = const.tile([128, B * NKT], i32)
        nc.sync.dma_start(ids_sb, ids.ap())
        padf = const.tile([128, B * NKT], f32)
        nc.vector.tensor_copy(padf, ids_sb)
        nc.vector.tensor_scalar_min(padf, padf, 1.0)

        ident = const.tile([128, 128], bf16)
        make_identity(nc, ident)
        # diagmask[x, y] = 1 if y >= x else 0  (keys on partitions, queries on free)
        diagmask = const.tile([128, 128], bf16)
        nc.gpsimd.memset(diagmask, 1.0)
        nc.gpsimd.affine_select(
            out=diagmask, in_=diagmask, compare_op=ALU.is_ge, fill=0.0,
            base=0, pattern=[[1, 128]], channel_multiplier=-1,
        )

        a2a_outs = []
        for b in range(B):
            # ---- projections for batch b ----
            # QT/KT: [128 dims(2 heads), 2048 rows]; v_aug: [128 keys, head, kt, 65]
            qt_sb = qkp.tile([128, S], bf16, name=f"qt{b}", tag="qt")
            kt_sb = qkp.tile([128, S], bf16, name=f"kt{b}", tag="kt")
            vaug = qkp.tile([128, 2, NKT, HD + 1], bf16, name=f"vaug{b}", tag="vaug")
            # Q/K computed directly in [dims, rows] layout (both heads: M=128)
            for ch in range(S // 512):
                rsl = slice(b * S + ch * 512, b * S + (ch + 1) * 512)
                csl = slice(ch * 512, (ch + 1) * 512)
                pqt = opool.tile([128, 512], f32, name="pqt", tag="o")
                pkt = opool.tile([128, 512], f32, name="pkt", tag="o")
                for c in range(NCH):
                    st = c == 0
                    sp = c == NCH - 1
                    rhs = xT_ch[c][:, rsl]
                    nc.tensor.matmul(pqt, wqT_sb[:, c, :], rhs, start=st, stop=sp)
                    nc.tensor.matmul(pkt, wkT_sb[:, c, :], rhs, start=st, stop=sp)
                nc.vector.tensor_scalar_add(qt_sb[:, csl], pqt, bq_col)
                nc.vector.tensor_scalar_add(kt_sb[:, csl], pkt, bk_col)
            # V in [keys, dims] layout for the PV matmul
            for m in range(NKT):
                rsl = slice(b * S + m * 128, b * S + (m + 1) * 128)
                pv = opool.tile([128, LD], f32, name="pv", tag="o")
                for c in range(NCH):
                    nc.tensor.matmul(pv, xT_ch[c][:, rsl], wvT_sb[:, c, :],
                                     start=(c == 0), stop=(c == NCH - 1))
                # bias, pad-zero rows, ones column (also pad-zeroed)
                tv = work.tile([128, LD], f32, name="tv", tag="tv")
                nc.vector.tensor_add(tv, pv, bv_bc)
                pcol = padf[:, b * NKT + m:b * NKT + m + 1]
                for h in range(2):
                    nc.vector.tensor_scalar_mul(
                        vaug[:, h, m, 0:HD], tv[:, h * HD:(h + 1) * HD], pcol)
                    nc.vector.tensor_copy(vaug[:, h, m, HD:HD + 1], pcol)

            # ---- attention for batch b, heads h=0,1 (local) ----
            stage = stg.tile([128, S], bf16, name=f"stage{b}", tag="stage")
            for h in range(2):
                hsl = slice(h * HD, (h + 1) * HD)
                ests = []

                def do_pv(m, h=h, hsl=hsl, vaug=vaug, stage=stage, ests=ests):
                    po = opool.tile([128, HD + 1], f32, name="po", tag="o")
                    for k2 in range(m + 1):
                        nc.tensor.matmul(
                            po,
                            ests[k2][:, (m - k2) * 128:(m - k2) * 128 + 128],
                            vaug[:, h, k2, :],
                            start=(k2 == 0), stop=(k2 == m))
                    rec = work.tile([128, 1], f32, name="rec", tag="rec")
                    nc.vector.reciprocal(rec, po[:, HD:HD + 1])
                    at = work.tile([128, HD], bf16, name="at", tag="at")
                    nc.vector.tensor_scalar_mul(at, po[:, 0:HD], rec)
                    pt = spool.tile([128, 128], bf16, name="pt", tag="s")
                    nc.tensor.transpose(pt[0:HD, :], at, ident)
                    nc.vector.tensor_copy(
                        stage[hsl, m * 128:(m + 1) * 128], pt[0:HD, :])

                for kt in range(NKT):
                    q0 = kt * 128          # first visible query
                    w = S - q0             # width of this kt row
                    e = est.tile([128, w], bf16, name=f"e{kt}", tag=f"e{kt}")
                    ests.append(e)
                    # scores in <=1024-wide chunks, exp each chunk
                    off = 0
                    while off < w:
                        cw = min(1024, w - off)
                        ps = spool.tile([128, 1024], f32, name="ps", tag="s")
                        o2 = 0
                        while o2 < cw:
                            mw = min(512, cw - o2)
                            nc.tensor.matmul(
                                ps[:, o2:o2 + mw],
                                kt_sb[hsl, kt * 128:(kt + 1) * 128],
                                qt_sb[hsl, q0 + off + o2:q0 + off + o2 + mw],
                                start=True, stop=True)
                            o2 += mw
                        nc.scalar.activation(
                            e[:, off:off + cw], ps[:, 0:cw], AF.Exp, scale=0.125)
                        off += cw
                    # causal mask on the diagonal 128 columns
                    nc.vector.tensor_mul(e[:, 0:128], e[:, 0:128], diagmask)
                    # PV shifted one kt behind scores so the tensor engine is
                    # never waiting on the exp it just requested
                    if kt >= 1:
                        do_pv(kt - 1)
                do_pv(NKT - 1)

            # ---- AllToAll for batch b, two q-half chunks ----
            # chunk t covers batch rows [t*1024, (t+1)*1024); each core ends
            # up with rows [t*1024 + core*128, +128) of this batch
            for t in range(2):
                a2a_in = dpool.tile([NCORES * 128, 128], bf16,
                                    name=f"a2ai{b}{t}", tag="a2ai", bufs=4)
                nc.sync.dma_start(
                    a2a_in.rearrange("(j p) r -> p j r", p=128),
                    stage[:, t * 1024:(t + 1) * 1024]
                    .rearrange("p (j r) -> p j r", j=NCORES))
                a2a_out = dpool.tile([NCORES * 128, 128], bf16,
                                     name=f"a2ao{b}{t}", tag="a2ao", bufs=4)
                nc.gpsimd.collective_compute(
                    "AllToAll", ALU.bypass,
                    replica_groups=[list(range(NCORES))],
                    ins=[a2a_in.opt()], outs=[a2a_out.opt()])
                a2a_outs.append((b, t, a2a_out))

        # ---- output projection (128-row chunks; b0 overlaps b1's A2As) ----
        for b, t, a2a_out in a2a_outs:
            a2a_sb = stg.tile([128, NCORES, 128], bf16, name=f"a2as{b}{t}",
                              tag="a2as", bufs=4)
            nc.sync.dma_start(
                a2a_sb, a2a_out.rearrange("(j p) r -> p j r", p=128))
            r0 = b * RB + t * 128
            for n in range(D // 512):
                pout = opool.tile([128, 512], f32, name="pout", tag="o")
                for c in range(NCH):
                    nc.tensor.matmul(
                        pout,
                        a2a_sb[:, c, :],
                        woT_sb[:, c, n * 512:(n + 1) * 512],
                        start=(c == 0), stop=(c == NCH - 1))
                ot = work.tile([128, 512], f32, name="ot", tag="ot")
                nc.vector.tensor_add(ot, pout, bo_bc[:, n * 512:(n + 1) * 512])
                nc.sync.dma_start(
                    out.ap()[r0:r0 + 128, n * 512:(n + 1) * 512], ot)

    nc.finalize()
    return nc


def _get_nc():
    with _lock:
        if "nc" not in _cache:
            _cache["nc"] = _build_nc()
        return _cache["nc"]


def _shard_inputs(x, input_ids, Wq, bq, Wk, bk, Wv, bv, Wo, bo):
    import ml_dtypes
    bf16 = ml_dtypes.bfloat16

    x = np.asarray(x, dtype=np.float32)
    xT = np.ascontiguousarray(x.reshape(R, D).T).astype(bf16)
    woT = np.ascontiguousarray(np.asarray(Wo, dtype=np.float32).T).astype(bf16)
    bo_f = np.asarray(bo, dtype=np.float32)
    ids = np.asarray(input_ids).astype(np.int32)
    # ids_r[p, b*NKT + t] = input_ids[b, t*128 + p]
    ids_r = np.ascontiguousarray(ids.reshape(B, NKT, 128).transpose(2, 0, 1)
                                 .reshape(128, B * NKT))
    Wq = np.asarray(Wq, dtype=np.float32)
    Wk = np.asarray(Wk, dtype=np.float32)
    Wv = np.asarray(Wv, dtype=np.float32)
    bq = np.asarray(bq, dtype=np.float32)
    bk = np.asarray(bk, dtype=np.float32)
    bv = np.asarray(bv, dtype=np.float32)

    in_maps = []
    for c in range(NCORES):
        sl = slice(c * LD, (c + 1) * LD)
        in_maps.append({
            "xT": xT,
            "wqT": np.ascontiguousarray(Wq[sl].T).astype(bf16),
            "wkT": np.ascontiguousarray(Wk[sl].T).astype(bf16),
            "wvT": np.ascontiguousarray(Wv[sl].T).astype(bf16),
            "woT": woT,
            "bq": bq[sl].copy(),
            "bk": bk[sl].copy(),
            "bv": bv[sl].copy(),
            "bo": bo_f,
            "ids": ids_r,
        })
    return in_maps


def run(trace=False, **inputs):
    """Run the kernel; returns (output, BassKernelResults)."""
    from concourse.bass_utils import run_bass_kernel_spmd

    nc = _get_nc()
    in_maps = _shard_inputs(**inputs)
    res = run_bass_kernel_spmd(nc, in_maps, core_ids=list(range(NCORES)),
                               trace=trace)
    full = np.empty((B, S, D), dtype=np.float32)
    for c in range(NCORES):
        o = np.asarray(res.results[c]["out"], dtype=np.float32)
        for b in range(B):
            for t in range(2):
                full[b, t * 1024 + c * 128:t * 1024 + (c + 1) * 128, :] = \
                    o[b * RB + t * 128:b * RB + (t + 1) * 128, :]
    return full, res


def kernel(**inputs) -> np.ndarray:
    full, _ = run(trace=False, **inputs)
    return full



# revision 2
# speedup vs baseline: 1.0156x; 1.0156x over previous
"""Trainium2 Bass kernel for a causal multi-head attention layer.

Model: b=2, s=2048, d_model=1024, 16 heads, head_dim=64, pad-index 0.
Sharding over 8 NeuronCores: each core owns 2 heads (128 of the 1024
attention dims) for both batches (head/tensor parallel).  After attention,
an AllToAll redistributes the per-head outputs so each core holds all 1024
attention dims for 1/8 of the sequence positions, where it runs the output
projection locally.  Output rows per core: 256 rows of each batch.

Attention is computed in 512-query stripes: per stripe, scores for both
heads run as concurrent row-group matmuls (h0 in PE rows 0-63, h1 in
64-127), one exp covers both heads, and the PV matmul keeps V stationary
(with a ones column for the softmax denominator) so the output lands
directly in [dims, queries] layout for the AllToAll.
"""

import threading

import numpy as np

B, S, D = 2, 2048, 1024
H, HD = 16, 64
NCORES = 8
LD = D // NCORES          # 128 local attention dims (2 heads)
R = B * S                 # 4096 flattened rows
RC = R // NCORES          # 512 output rows per core
RB = S // NCORES          # 256 rows per batch per core
NKT = S // 128            # 16 key tiles per batch
NCH = D // 128            # 8 contraction chunks of d_model
NST = S // 512            # 4 query stripes per batch

_cache = {}
_lock = threading.Lock()


def _stripe_layout():
    """Per stripe c: list of (kt, width, q_start, offset-in-block), block len."""
    layout = []
    for c in range(NST):
        entries = []
        off = 0
        for kt in range(4 * c + 4):
            qs = max(512 * c, kt * 128)
            w = 512 * (c + 1) - qs
            entries.append((kt, w, qs, off))
            off += w
        layout.append((entries, off))
    return layout


def _build_nc():
    import concourse.mybir as mybir
    import concourse.tile as tile
    from concourse import bacc
    from contextlib import ExitStack

    f32 = mybir.dt.float32
    bf16 = mybir.dt.bfloat16
    i32 = mybir.dt.int32
    AF = mybir.ActivationFunctionType
    ALU = mybir.AluOpType

    nc = bacc.Bacc(None, target_bir_lowering=False, num_devices=NCORES)

    xT = nc.declare_dram_parameter("xT", [D, R], bf16, isOutput=False)
    wqT = nc.declare_dram_parameter("wqT", [D, LD], bf16, isOutput=False)
    wkT = nc.declare_dram_parameter("wkT", [D, LD], bf16, isOutput=False)
    wvT = nc.declare_dram_parameter("wvT", [D, LD], bf16, isOutput=False)
    woT = nc.declare_dram_parameter("woT", [D, D], bf16, isOutput=False)
    bq = nc.declare_dram_parameter("bq", [LD], f32, isOutput=False)
    bk = nc.declare_dram_parameter("bk", [LD], f32, isOutput=False)
    bv = nc.declare_dram_parameter("bv", [LD], f32, isOutput=False)
    bo = nc.declare_dram_parameter("bo", [D], f32, isOutput=False)
    ids = nc.declare_dram_parameter("ids", [128, B * NKT], i32, isOutput=False)
    out = nc.declare_dram_parameter("out", [RC, D], f32, isOutput=True)

    layout = _stripe_layout()

    with ExitStack() as ctx:
        tc = ctx.enter_context(tile.TileContext(nc))
        const = ctx.enter_context(tc.tile_pool(name="const", bufs=1))
        xcp = ctx.enter_context(tc.tile_pool(name="xcp", bufs=1))
        qkp = ctx.enter_context(tc.tile_pool(name="qkp", bufs=2))
        estp = ctx.enter_context(tc.tile_pool(name="estp", bufs=1))
        stg = ctx.enter_context(tc.tile_pool(name="stg", bufs=2))
        work = ctx.enter_context(tc.tile_pool(name="work", bufs=2))
        recp = ctx.enter_context(tc.tile_pool(name="recp", bufs=2))
        spool = ctx.enter_context(tc.tile_pool(name="spool", bufs=2, space="PSUM"))
        pvpool = ctx.enter_context(tc.tile_pool(name="pvpool", bufs=1, space="PSUM"))
        ppool = ctx.enter_context(tc.tile_pool(name="ppool", bufs=2, space="PSUM"))
        dpool = ctx.enter_context(tc.tile_pool(name="dram", bufs=2, space="DRAM"))

        # ---- constants (small weights first so compute can start early) ----
        wqT_sb = const.tile([128, NCH, LD], bf16)
        nc.sync.dma_start(wqT_sb, wqT.ap().rearrange("(c p) d -> p c d", p=128))
        wkT_sb = const.tile([128, NCH, LD], bf16)
        nc.sync.dma_start(wkT_sb, wkT.ap().rearrange("(c p) d -> p c d", p=128))
        wvT_sb = const.tile([128, NCH, LD], bf16)
        nc.sync.dma_start(wvT_sb, wvT.ap().rearrange("(c p) d -> p c d", p=128))

        bq_col = const.tile([128, 1], f32)
        nc.sync.dma_start(bq_col, bq.ap().rearrange("(p o) -> p o", o=1))
        bk_col = const.tile([128, 1], f32)
        nc.sync.dma_start(bk_col, bk.ap().rearrange("(p o) -> p o", o=1))
        bv_bc = const.tile([128, LD], f32)
        nc.sync.dma_start(bv_bc, bv.ap().partition_broadcast(128))

        ids_sb = const.tile([128, B * NKT], i32)
        nc.sync.dma_start(ids_sb, ids.ap())

        # x^T for batch 0, chunked by (row-block, dim-chunk) so projection
        # matmuls start as soon as the first row block lands
        xTr = xT.ap().rearrange("(c p) r -> c p r", p=128)
        xc = [xcp.tile([128, S], bf16, name=f"xc{c}", tag=f"xc{c}")
              for c in range(NCH)]
        for rb in range(4):
            rsl = slice(rb * 512, (rb + 1) * 512)
            for c in range(NCH):
                nc.sync.dma_start(xc[c][:, rsl], xTr[c][:, rsl])

        woT_sb = const.tile([128, NCH, D], bf16)
        nc.sync.dma_start(woT_sb, woT.ap().rearrange("(c p) n -> p c n", p=128))
        bo_bc = const.tile([128, D], f32)
        nc.sync.dma_start(bo_bc, bo.ap().partition_broadcast(128))

        padf = const.tile([128, B * NKT], f32)
        nc.vector.tensor_copy(padf, ids_sb)
        nc.vector.tensor_scalar_min(padf, padf, 1.0)

        # diagmask2[x, h, y] = 1 if y >= x else 0 (keys on partitions)
        diagmask = const.tile([128, 128], bf16)
        nc.gpsimd.memset(diagmask, 1.0)
        nc.gpsimd.affine_select(
            out=diagmask, in_=diagmask, compare_op=ALU.is_ge, fill=0.0,
            base=0, pattern=[[1, 128]], channel_multiplier=-1,
        )
        diagmask2 = const.tile([128, 2, 128], bf16)
        nc.vector.tensor_copy(diagmask2[:, 0, :], diagmask)
        nc.vector.tensor_copy(diagmask2[:, 1, :], diagmask)

        a2a_outs = []
        for b in range(B):
            if b > 0:
                # batch 1 x^T overwrites batch 0's chunks (WAR-tracked)
                for rb in range(4):
                    rsl = slice(rb * 512, (rb + 1) * 512)
                    dsl = slice(b * S + rb * 512, b * S + (rb + 1) * 512)
                    for c in range(NCH):
                        nc.sync.dma_start(xc[c][:, rsl], xTr[c][:, dsl])

            # ---- Q/K projections: [dims, rows], both heads stacked ----
            qt_sb = qkp.tile([128, S], bf16, name=f"qt{b}", tag="qt")
            kt_sb = qkp.tile([128, S], bf16, name=f"kt{b}", tag="kt")
            for rb in range(4):
                rsl = slice(rb * 512, (rb + 1) * 512)
                pqt = ppool.tile([128, 512], f32, name="pqt", tag="pp")
                pkt = ppool.tile([128, 512], f32, name="pkt", tag="pp")
                for c in range(NCH):
                    st = c == 0
                    sp = c == NCH - 1
                    rhs = xc[c][:, rsl]
                    nc.tensor.matmul(pqt, wqT_sb[:, c, :], rhs, start=st, stop=sp)
                    nc.tensor.matmul(pkt, wkT_sb[:, c, :], rhs, start=st, stop=sp)
                nc.vector.tensor_scalar_add(qt_sb[:, rsl], pqt, bq_col)
                nc.vector.tensor_scalar_add(kt_sb[:, rsl], pkt, bk_col)

            # ---- V: [keys, dims] with ones column (pad-masked) ----
            vaug = qkp.tile([128, 2, NKT, HD + 1], bf16, name=f"vaug{b}",
                            tag="vaug")
            for m in range(NKT):
                msl = slice(m * 128, (m + 1) * 128)
                pv = ppool.tile([128, LD], f32, name="pv", tag="pp")
                for c in range(NCH):
                    nc.tensor.matmul(pv, xc[c][:, msl], wvT_sb[:, c, :],
                                     start=(c == 0), stop=(c == NCH - 1))
                tv = work.tile([128, LD], f32, name="tv", tag="tv")
                nc.vector.tensor_add(tv, pv, bv_bc)
                pcol = padf[:, b * NKT + m:b * NKT + m + 1]
                for h in range(2):
                    nc.vector.tensor_scalar_mul(
                        vaug[:, h, m, 0:HD], tv[:, h * HD:(h + 1) * HD], pcol)
                    nc.vector.tensor_copy(vaug[:, h, m, HD:HD + 1], pcol)

            # ---- attention in 512-query stripes ----
            stage = stg.tile([128, S], bf16, name=f"stage{b}", tag="stage")
            ests = [estp.tile([128, 2, blocklen], bf16, name=f"est{c}",
                              tag=f"est{c}")
                    for c, (_, blocklen) in enumerate(layout)]

            def do_scores(c, b=b, qt_sb=qt_sb, kt_sb=kt_sb, ests=ests):
                entries, _ = layout[c]
                est = ests[c]
                for kt, w, qs, off in entries:
                    ksl = slice(kt * 128, (kt + 1) * 128)
                    ps = spool.tile([128, 2, 512], f32, name="ps", tag="ps")
                    # both heads run concurrently in separate PE row groups
                    nc.tensor.matmul(ps[:, 0, 0:w], kt_sb[0:64, ksl],
                                     qt_sb[0:64, qs:qs + w],
                                     start=True, stop=True)
                    nc.tensor.matmul(ps[:, 1, 0:w], kt_sb[64:128, ksl],
                                     qt_sb[64:128, qs:qs + w],
                                     start=True, stop=True)
                    nc.scalar.activation(est[:, :, off:off + w], ps[:, :, 0:w],
                                         AF.Exp, scale=0.125)
                    if kt >= 4 * c:  # diagonal tile: causal mask
                        nc.vector.tensor_mul(est[:, :, off:off + 128],
                                             est[:, :, off:off + 128],
                                             diagmask2)

            def do_pv(c, b=b, vaug=vaug, stage=stage, ests=ests):
                entries, _ = layout[c]
                est = ests[c]
                for h in range(2):
                    po = pvpool.tile([HD + 1, 512], f32, name=f"po{h}",
                                     tag=f"po{h}")
                    last = 4 * c + 3
                    for kt, w, qs, off in entries:
                        po_off = qs - 512 * c
                        nc.tensor.matmul(po[:, po_off:po_off + w],
                                         vaug[:, h, kt, :],
                                         est[:, h, off:off + w],
                                         start=(kt == 0), stop=(kt == last))
                    rec = recp.tile([1, 512], f32, name="rec", tag="rec")
                    nc.vector.reciprocal(rec, po[HD:HD + 1, :])
                    rbc = recp.tile([HD, 512], f32, name="rbc", tag="rbc")
                    nc.gpsimd.partition_broadcast(rbc, rec)
                    nc.vector.tensor_mul(
                        stage[h * HD:(h + 1) * HD, 512 * c:512 * (c + 1)],
                        po[0:HD, :], rbc)

            def do_a2a(t, b=b, stage=stage):
                a2a_in = dpool.tile([NCORES * 128, 128], bf16,
                                    name=f"a2ai{b}{t}", tag="a2ai", bufs=4)
                nc.sync.dma_start(
                    a2a_in.rearrange("(j p) r -> p j r", p=128),
                    stage[:, t * 1024:(t + 1) * 1024]
                    .rearrange("p (j r) -> p j r", j=NCORES))
                a2a_out = dpool.tile([NCORES * 128, 128], bf16,
                                     name=f"a2ao{b}{t}", tag="a2ao", bufs=4)
                nc.gpsimd.collective_compute(
                    "AllToAll", ALU.bypass,
                    replica_groups=[list(range(NCORES))],
                    ins=[a2a_in.opt()], outs=[a2a_out.opt()])
                a2a_outs.append((b, t, a2a_out))

            # PV trails scores by one stripe so the tensor engine never
            # waits on the exp it just requested
            do_scores(0)
            do_scores(1)
            do_pv(0)
            do_scores(2)
            do_pv(1)
            do_a2a(0)
            do_scores(3)
            do_pv(2)
            do_pv(3)
            do_a2a(1)

        # ---- output projection (after both batches; overlaps tail A2As) ----
        for b, t, a2a_out in a2a_outs:
            a2a_sb = stg.tile([128, NCORES, 128], bf16, name=f"a2as{b}{t}",
                              tag="a2as", bufs=4)
            nc.sync.dma_start(
                a2a_sb, a2a_out.rearrange("(j p) r -> p j r", p=128))
            r0 = b * RB + t * 128
            for n in range(D // 512):
                pout = ppool.tile([128, 512], f32, name="pout", tag="pp")
                for c in range(NCH):
                    nc.tensor.matmul(
                        pout,
                        a2a_sb[:, c, :],
                        woT_sb[:, c, n * 512:(n + 1) * 512],
                        start=(c == 0), stop=(c == NCH - 1))
                ot = work.tile([128, 512], f32, name="ot", tag="ot")
                nc.vector.tensor_add(ot, pout, bo_bc[:, n * 512:(n + 1) * 512])
                nc.sync.dma_start(
                    out.ap()[r0:r0 + 128, n * 512:(n + 1) * 512], ot)

    nc.finalize()
    return nc


def _get_nc():
    with _lock:
        if "nc" not in _cache:
            _cache["nc"] = _build_nc()
        return _cache["nc"]


def _shard_inputs(x, input_ids, Wq, bq, Wk, bk, Wv, bv, Wo, bo):
    import ml_dtypes
    bf16 = ml_dtypes.bfloat16

    x = np.asarray(x, dtype=np.float32)
    xT = np.ascontiguousarray(x.reshape(R, D).T).astype(bf16)
    woT = np.ascontiguousarray(np.asarray(Wo, dtype=np.float32).T).astype(bf16)
    bo_f = np.asarray(bo, dtype=np.float32)
    ids = np.asarray(input_ids).astype(np.int32)
    # ids_r[p, b*NKT + t] = input_ids[b, t*128 + p]
    ids_r = np.ascontiguousarray(ids.reshape(B, NKT, 128).transpose(2, 0, 1)
                                 .reshape(128, B * NKT))
    Wq = np.asarray(Wq, dtype=np.float32)
    Wk = np.asarray(Wk, dtype=np.float32)
    Wv = np.asarray(Wv, dtype=np.float32)
    bq = np.asarray(bq, dtype=np.float32)
    bk = np.asarray(bk, dtype=np.float32)
    bv = np.asarray(bv, dtype=np.float32)

    in_maps = []
    for c in range(NCORES):
        sl = slice(c * LD, (c + 1) * LD)
        in_maps.append({
            "xT": xT,
            "wqT": np.ascontiguousarray(Wq[sl].T).astype(bf16),
            "wkT": np.ascontiguousarray(Wk[sl].T).astype(bf16),
            "wvT": np.ascontiguousarray(Wv[sl].T).astype(bf16),
            "woT": woT,
            "bq": bq[sl].copy(),
            "bk": bk[sl].copy(),
            "bv": bv[sl].copy(),
            "bo": bo_f,
            "ids": ids_r,
        })
    return in_maps


def run(trace=False, **inputs):
    """Run the kernel; returns (output, BassKernelResults)."""
    from concourse.bass_utils import run_bass_kernel_spmd

    nc = _get_nc()
    in_maps = _shard_inputs(**inputs)
    res = run_bass_kernel_spmd(nc, in_maps, core_ids=list(range(NCORES)),
                               trace=trace)
    full = np.empty((B, S, D), dtype=np.float32)
    for c in range(NCORES):
        o = np.asarray(res.results[c]["out"], dtype=np.float32)
        for b in range(B):
            for t in range(2):
                full[b, t * 1024 + c * 128:t * 1024 + (c + 1) * 128, :] = \
                    o[b * RB + t * 128:b * RB + (t + 1) * 128, :]
    return full, res


def kernel(**inputs) -> np.ndarray:
    full, _ = run(trace=False, **inputs)
    return full


# revision 12
# speedup vs baseline: 1.5629x; 1.5389x over previous
"""Trainium2 Bass kernel for a causal multi-head attention layer.

Model: b=2, s=2048, d_model=1024, 16 heads, head_dim=64, pad-index 0.
Sharding over 8 NeuronCores: each core owns 2 heads (128 of the 1024
attention dims) for both batches (head/tensor parallel).  After attention,
an AllToAll redistributes the per-head outputs so each core holds all 1024
attention dims for 1/8 of the sequence positions, where it runs the output
projection locally.  Output rows per core: 256 rows of each batch.

Attention is computed in 512-query stripes: per stripe, scores for both
heads run as concurrent row-group matmuls (h0 in PE rows 0-63, h1 in
64-127), one exp covers both heads, and the PV matmul keeps V stationary
(with a ones column for the softmax denominator) so the output lands
directly in [dims, queries] layout for the AllToAll.
"""

import threading

import numpy as np

B, S, D = 2, 2048, 1024
H, HD = 16, 64
NCORES = 8
LD = D // NCORES          # 128 local attention dims (2 heads)
R = B * S                 # 4096 flattened rows
RC = R // NCORES          # 512 output rows per core
RB = S // NCORES          # 256 rows per batch per core
NKT = S // 128            # 16 key tiles per batch
NCH = D // 128            # 8 contraction chunks of d_model
NST = S // 512            # 4 query stripes per batch

_cache = {}
_lock = threading.Lock()


def _stripe_layout():
    """Per stripe c: list of (kt, width, q_start, offset-in-block), block len."""
    layout = []
    for c in range(NST):
        entries = []
        off = 0
        for kt in range(4 * c + 4):
            qs = max(512 * c, kt * 128)
            w = 512 * (c + 1) - qs
            entries.append((kt, w, qs, off))
            off += w
        layout.append((entries, off))
    return layout


def _build_nc():
    import concourse.mybir as mybir
    import concourse.tile as tile
    from concourse import bacc
    from contextlib import ExitStack

    f32 = mybir.dt.float32
    bf16 = mybir.dt.bfloat16
    i32 = mybir.dt.int32
    AF = mybir.ActivationFunctionType
    ALU = mybir.AluOpType

    nc = bacc.Bacc(None, target_bir_lowering=False, num_devices=NCORES)

    xT = nc.declare_dram_parameter("xT", [D, R], bf16, isOutput=False)
    wqT = nc.declare_dram_parameter("wqT", [D, LD], bf16, isOutput=False)
    wkT = nc.declare_dram_parameter("wkT", [D, LD], bf16, isOutput=False)
    wvT = nc.declare_dram_parameter("wvT", [D, LD], bf16, isOutput=False)
    woT = nc.declare_dram_parameter("woT", [D, D], bf16, isOutput=False)
    bq = nc.declare_dram_parameter("bq", [LD], f32, isOutput=False)
    bk = nc.declare_dram_parameter("bk", [LD], f32, isOutput=False)
    bv = nc.declare_dram_parameter("bv", [LD], f32, isOutput=False)
    bo = nc.declare_dram_parameter("bo", [D], f32, isOutput=False)
    ids = nc.declare_dram_parameter("ids", [128, B * NKT], i32, isOutput=False)
    out = nc.declare_dram_parameter("out", [RC, D], f32, isOutput=True)

    layout = _stripe_layout()

    with ExitStack() as ctx:
        tc = ctx.enter_context(tile.TileContext(nc))
        const = ctx.enter_context(tc.tile_pool(name="const", bufs=1))
        xcp = ctx.enter_context(tc.tile_pool(name="xcp", bufs=1))
        qkp = ctx.enter_context(tc.tile_pool(name="qkp", bufs=2))
        estp = ctx.enter_context(tc.tile_pool(name="estp", bufs=1))
        stg = ctx.enter_context(tc.tile_pool(name="stg", bufs=2))
        work = ctx.enter_context(tc.tile_pool(name="work", bufs=2))
        recp = ctx.enter_context(tc.tile_pool(name="recp", bufs=1))
        spool = ctx.enter_context(tc.tile_pool(name="spool", bufs=2, space="PSUM"))
        pvpool = ctx.enter_context(tc.tile_pool(name="pvpool", bufs=1, space="PSUM"))
        ppool = ctx.enter_context(tc.tile_pool(name="ppool", bufs=2, space="PSUM"))
        dpool = ctx.enter_context(tc.tile_pool(name="dram", bufs=2, space="DRAM"))

        # ---- constants (small weights first so compute can start early) ----
        wqT_sb = const.tile([128, NCH, LD], bf16)
        nc.sync.dma_start(wqT_sb, wqT.ap().rearrange("(c p) d -> p c d", p=128))
        wkT_sb = const.tile([128, NCH, LD], bf16)
        nc.sync.dma_start(wkT_sb, wkT.ap().rearrange("(c p) d -> p c d", p=128))
        wvT_sb = const.tile([128, NCH, LD], bf16)
        nc.sync.dma_start(wvT_sb, wvT.ap().rearrange("(c p) d -> p c d", p=128))

        bq_col = const.tile([128, 1], f32)
        nc.sync.dma_start(bq_col, bq.ap().rearrange("(p o) -> p o", o=1))
        bk_col = const.tile([128, 1], f32)
        nc.sync.dma_start(bk_col, bk.ap().rearrange("(p o) -> p o", o=1))
        bv_bc = const.tile([128, LD], f32)
        nc.sync.dma_start(bv_bc, bv.ap().partition_broadcast(128))

        ids_sb = const.tile([128, B * NKT], i32)
        nc.sync.dma_start(ids_sb, ids.ap())
        ones64 = const.tile([1, 64], bf16)
        nc.vector.memset(ones64, 1.0)

        # x^T for batch 0, chunked by (row-block, dim-chunk) so projection
        # matmuls start as soon as the first row block lands
        xTr = xT.ap().rearrange("(c p) r -> c p r", p=128)
        xc = [xcp.tile([128, S], bf16, name=f"xc{c}", tag=f"xc{c}")
              for c in range(NCH)]
        for rb in range(4):
            rsl = slice(rb * 512, (rb + 1) * 512)
            for c in range(NCH):
                nc.sync.dma_start(xc[c][:, rsl], xTr[c][:, rsl])

        woT_sb = const.tile([128, NCH, D], bf16)
        nc.sync.dma_start(woT_sb, woT.ap().rearrange("(c p) n -> p c n", p=128))
        bo_bc = const.tile([128, D], f32)
        nc.sync.dma_start(bo_bc, bo.ap().partition_broadcast(128))

        padf = const.tile([128, B * NKT], f32)
        nc.vector.tensor_copy(padf, ids_sb)
        nc.vector.tensor_scalar_min(padf, padf, 1.0)

        # diagmask2[x, h, y] = 1 if y >= x else 0 (keys on partitions)
        diagmask = const.tile([128, 128], bf16)
        nc.gpsimd.memset(diagmask, 1.0)
        nc.gpsimd.affine_select(
            out=diagmask, in_=diagmask, compare_op=ALU.is_ge, fill=0.0,
            base=0, pattern=[[1, 128]], channel_multiplier=-1,
        )
        diagmask2 = const.tile([128, 2, 128], bf16)
        nc.vector.tensor_copy(diagmask2[:, 0, :], diagmask)
        nc.vector.tensor_copy(diagmask2[:, 1, :], diagmask)

        a2a_outs = []
        for b in range(B):
            # ---- Q/K projections: [dims, rows], both heads stacked ----
            qt_sb = qkp.tile([128, S], bf16, name=f"qt{b}", tag="qt")
            kt_sb = qkp.tile([128, S], bf16, name=f"kt{b}", tag="kt")
            for rb in range(4):
                rsl = slice(rb * 512, (rb + 1) * 512)
                pqt = ppool.tile([128, 512], f32, name="pqt", tag="pp")
                pkt = ppool.tile([128, 512], f32, name="pkt", tag="pp")
                for c in range(NCH):
                    st = c == 0
                    sp = c == NCH - 1
                    rhs = xc[c][:, rsl]
                    nc.tensor.matmul(pqt, wqT_sb[:, c, :], rhs, start=st, stop=sp)
                    nc.tensor.matmul(pkt, wkT_sb[:, c, :], rhs, start=st, stop=sp)
                nc.vector.tensor_scalar_add(qt_sb[:, rsl], pqt, bq_col)
                nc.vector.tensor_scalar_add(kt_sb[:, rsl], pkt, bk_col)

            # ---- V: [keys, dims] with ones column (pad-masked) ----
            vaug = qkp.tile([128, 2, NKT, HD + 1], bf16, name=f"vaug{b}",
                            tag="vaug")
            for m in range(NKT):
                msl = slice(m * 128, (m + 1) * 128)
                pv = ppool.tile([128, LD], f32, name="pv", tag="pp")
                for c in range(NCH):
                    nc.tensor.matmul(pv, xc[c][:, msl], wvT_sb[:, c, :],
                                     start=(c == 0), stop=(c == NCH - 1))
                tv = work.tile([128, LD], f32, name="tv", tag="tv")
                nc.vector.tensor_add(tv, pv, bv_bc)
                pcol = padf[:, b * NKT + m:b * NKT + m + 1]
                for h in range(2):
                    nc.vector.tensor_scalar_mul(
                        vaug[:, h, m, 0:HD], tv[:, h * HD:(h + 1) * HD], pcol)
                    nc.vector.tensor_copy(vaug[:, h, m, HD:HD + 1], pcol)

            # next batch's x^T load starts now (overlaps this batch's
            # attention; WAR on this batch's projection reads is tracked)
            if b + 1 < B:
                for rb in range(4):
                    rsl = slice(rb * 512, (rb + 1) * 512)
                    dsl = slice((b + 1) * S + rb * 512,
                                (b + 1) * S + (rb + 1) * 512)
                    for c in range(NCH):
                        nc.sync.dma_start(xc[c][:, rsl], xTr[c][:, dsl])

            # ---- attention in 512-query stripes ----
            stage = stg.tile([128, S], bf16, name=f"stage{b}", tag="stage")
            ests = [estp.tile([128, 2, blocklen], bf16, name=f"est{c}",
                              tag=f"est{c}")
                    for c, (_, blocklen) in enumerate(layout)]

            def do_scores(c, b=b, qt_sb=qt_sb, kt_sb=kt_sb, ests=ests):
                entries, _ = layout[c]
                est = ests[c]
                for kt, w, qs, off in entries:
                    ksl = slice(kt * 128, (kt + 1) * 128)
                    ps = spool.tile([128, 2, 512], f32, name="ps", tag="ps")
                    # both heads run concurrently in separate PE row groups
                    nc.tensor.matmul(ps[:, 0, 0:w], kt_sb[0:64, ksl],
                                     qt_sb[0:64, qs:qs + w],
                                     start=True, stop=True)
                    nc.tensor.matmul(ps[:, 1, 0:w], kt_sb[64:128, ksl],
                                     qt_sb[64:128, qs:qs + w],
                                     start=True, stop=True)
                    nc.scalar.activation(est[:, :, off:off + w], ps[:, :, 0:w],
                                         AF.Exp, scale=0.125)
                    if kt >= 4 * c:  # diagonal tile: causal mask
                        nc.vector.tensor_mul(est[:, :, off:off + 128],
                                             est[:, :, off:off + 128],
                                             diagmask2)

            pos = {}

            def do_pv(c, b=b, vaug=vaug, ests=ests, pos=pos):
                entries, _ = layout[c]
                est = ests[c]
                for h in range(2):
                    po = pvpool.tile([128, 512], f32, name=f"po{h}",
                                     tag=f"po{h}")
                    pos[(c, h)] = po
                    last = 4 * c + 3
                    for kt, w, qs, off in entries:
                        po_off = qs - 512 * c
                        nc.tensor.matmul(po[0:HD + 1, po_off:po_off + w],
                                         vaug[:, h, kt, :],
                                         est[:, h, off:off + w],
                                         start=(kt == 0), stop=(kt == last))

            def do_div(c, stage=stage, pos=pos):
                # softmax division: recip of the ones-column row, broadcast
                # to 64 partitions via a tiny PE matmul into po's upper half
                recs = []
                for h in range(2):
                    po = pos[(c, h)]
                    den = recp.tile([1, 512], f32, name="den", tag=f"den{h}")
                    # custom-DVE recip ignores the input base partition, so
                    # stage the denominator row at partition 0 first
                    nc.vector.tensor_copy(den, po[HD:HD + 1, :])
                    rec = recp.tile([1, 512], f32, name="rec", tag=f"rec{h}")
                    nc.vector.reciprocal_approx_fast(rec, den)
                    recb = recp.tile([1, 512], bf16, name="recb",
                                     tag=f"recb{h}")
                    nc.vector.tensor_copy(recb, rec)
                    recs.append(recb)
                for h in range(2):
                    nc.tensor.matmul(pos[(c, h)][64:128, :], ones64, recs[h],
                                     start=True, stop=True,
                                     skip_group_check=True)
                for h in range(2):
                    po = pos[(c, h)]
                    rbc = recp.tile([HD, 512], bf16, name="rbc", tag=f"rbc{h}")
                    nc.vector.tensor_copy(rbc, po[64:128, :])
                    nc.vector.tensor_mul(
                        stage[h * HD:(h + 1) * HD, 512 * c:512 * (c + 1)],
                        po[0:HD, :], rbc)

            def do_a2a(b=b, stage=stage):
                a2a_in = dpool.tile([NCORES * 128, RB], bf16,
                                    name=f"a2ai{b}", tag="a2ai", bufs=2)
                nc.gpsimd.dma_start(
                    a2a_in.rearrange("(j p) r -> p j r", p=128),
                    stage.rearrange("p (j r) -> p j r", j=NCORES))
                a2a_out = dpool.tile([NCORES * 128, RB], bf16,
                                     name=f"a2ao{b}", tag="a2ao", bufs=2)
                nc.gpsimd.collective_compute(
                    "AllToAll", ALU.bypass,
                    replica_groups=[list(range(NCORES))],
                    ins=[a2a_in.opt()], outs=[a2a_out.opt()])
                a2a_outs.append((b, a2a_out))

            # PV trails scores by one stripe; divisions trail by one more so
            # the broadcast matmuls never stall the PE FIFO
            do_scores(0)
            do_scores(1)
            do_pv(0)
            do_scores(2)
            do_div(0)
            do_pv(1)
            do_scores(3)
            do_div(1)
            do_pv(2)
            do_div(2)
            do_pv(3)
            do_div(3)
            do_a2a()

        # ---- output projection (after both batches; overlaps tail A2A) ----
        for b, a2a_out in a2a_outs:
            a2a_sb = stg.tile([128, NCORES, RB], bf16, name=f"a2as{b}",
                              tag="a2as", bufs=2)
            nc.scalar.dma_start(
                a2a_sb, a2a_out.rearrange("(j p) r -> p j r", p=128))
            for rc in range(RB // 128):
                r0 = b * RB + rc * 128
                rsl = slice(rc * 128, (rc + 1) * 128)
                for n in range(D // 512):
                    pout = spool.tile([128, 512], f32, name="pout", tag="ps")
                    for c in range(NCH):
                        nc.tensor.matmul(
                            pout,
                            a2a_sb[:, c, rsl],
                            woT_sb[:, c, n * 512:(n + 1) * 512],
                            start=(c == 0), stop=(c == NCH - 1))
                    ot = work.tile([128, 512], f32, name="ot", tag="ot")
                    nc.vector.tensor_add(ot, pout,
                                         bo_bc[:, n * 512:(n + 1) * 512])
                    nc.sync.dma_start(
                        out.ap()[r0:r0 + 128, n * 512:(n + 1) * 512], ot)

    nc.finalize()
    return nc


def _get_nc():
    with _lock:
        if "nc" not in _cache:
            _cache["nc"] = _build_nc()
        return _cache["nc"]


def _shard_inputs(x, input_ids, Wq, bq, Wk, bk, Wv, bv, Wo, bo):
    import ml_dtypes
    bf16 = ml_dtypes.bfloat16

    x = np.asarray(x, dtype=np.float32)
    xT = np.ascontiguousarray(x.reshape(R, D).T).astype(bf16)
    woT = np.ascontiguousarray(np.asarray(Wo, dtype=np.float32).T).astype(bf16)
    bo_f = np.asarray(bo, dtype=np.float32)
    ids = np.asarray(input_ids).astype(np.int32)
    # ids_r[p, b*NKT + t] = input_ids[b, t*128 + p]
    ids_r = np.ascontiguousarray(ids.reshape(B, NKT, 128).transpose(2, 0, 1)
                                 .reshape(128, B * NKT))
    Wq = np.asarray(Wq, dtype=np.float32)
    Wk = np.asarray(Wk, dtype=np.float32)
    Wv = np.asarray(Wv, dtype=np.float32)
    bq = np.asarray(bq, dtype=np.float32)
    bk = np.asarray(bk, dtype=np.float32)
    bv = np.asarray(bv, dtype=np.float32)

    in_maps = []
    for c in range(NCORES):
        sl = slice(c * LD, (c + 1) * LD)
        in_maps.append({
            "xT": xT,
            "wqT": np.ascontiguousarray(Wq[sl].T).astype(bf16),
            "wkT": np.ascontiguousarray(Wk[sl].T).astype(bf16),
            "wvT": np.ascontiguousarray(Wv[sl].T).astype(bf16),
            "woT": woT,
            "bq": bq[sl].copy(),
            "bk": bk[sl].copy(),
            "bv": bv[sl].copy(),
            "bo": bo_f,
            "ids": ids_r,
        })
    return in_maps


def run(trace=False, **inputs):
    """Run the kernel; returns (output, BassKernelResults)."""
    from concourse.bass_utils import run_bass_kernel_spmd

    nc = _get_nc()
    in_maps = _shard_inputs(**inputs)
    res = run_bass_kernel_spmd(nc, in_maps, core_ids=list(range(NCORES)),
                               trace=trace)
    full = np.empty((B, S, D), dtype=np.float32)
    for c in range(NCORES):
        o = np.asarray(res.results[c]["out"], dtype=np.float32)
        for b in range(B):
            full[b, c * RB:(c + 1) * RB, :] = o[b * RB:(b + 1) * RB, :]
    return full, res


def kernel(**inputs) -> np.ndarray:
    full, _ = run(trace=False, **inputs)
    return full
